# revision 1
# baseline (speedup 1.0000x reference)
"""Trainium2 Bass kernel for nn_Conv_12962211300035 (e3nn-style GNN conv).

Math (per edge e, src s, dst d):
  x = node_attr[s]; x0 = x[:16], x1 = x[16:].reshape(16,3)
  h = silu(edge_attr @ w1 + b1); W = (h @ w2 + b2).reshape(4,16,16)
  out0[w]   = sum_u (x0*y0)[u] W0[u,w] + (1/sqrt3) sum_u (sum_i x1[u,i]y1[i]) W1[u,w]
  out1[w,i] = sum_u (x0[u]*y1[i]) W2[u,w] + sum_u (x1[u,i]*y0) W3[u,w]
  out_ij = alpha * [out0 | out1.reshape(48)]     (alpha = 1/sqrt(32))
  node output = scatter-mean of out_ij over dst.

Kernel strategy (8 NeuronCores, SPMD, no collectives):
  - Nodes range-partitioned: core c owns nodes [c*1664, (c+1)*1664); edges are
    assigned to the core owning their dst, sorted by dst, grouped into 13
    windows of 128 nodes, tiled into 128-edge tiles (window tile counts
    uniform across cores; padding edges have out-of-range dst -> one-hot 0).
  - Per tile: gather x (indirect DMA from DRAM, bf16), build a 128-dim
    geometry feature F = [z0|q0|q1|q2|r0|r1|r2|z1] on DVE, transpose via PE,
    A = F.T-matmul against a fixed [128, 64*18] tensor (w2/b2/CG folded in),
    P = A * h'-broadcast (DVE), scatter-matmul with a one-hot of dst into a
    per-window PSUM accumulator [128 nodes, 64*18+1].
  - Per window: copy accumulator out, reduce over k (18), divide by counts.
"""

import os
import sys

sys.path.insert(0, "/opt/trn_rl_repo")

import numpy as np
import ml_dtypes
from contextlib import ExitStack

import concourse.bass as bass
import concourse.bacc as bacc
import concourse.tile as tile
import concourse.mybir as mybir
from concourse.bass import IndirectOffsetOnAxis
from concourse.bass_utils import run_bass_kernel_spmd
from concourse.masks import make_identity

MUL = 16
N_NODES = 12500
N_EDGES = 200000
EA = 16
NCORES = 8
NWIN = 13          # windows of 128 nodes per core
NR = NWIN * 128    # 1664 nodes per core (8*1664 = 13312 >= 12500)
NK = 18            # h' slots: 16 h + 1 bias + 1 zero pad (even for DVE 2x)
NO = 64            # outputs per edge
NA = NO * NK       # 1152 A/P columns
PAD_DST = 999.0

F32 = mybir.dt.float32
BF16 = mybir.dt.bfloat16
I32 = mybir.dt.int32
BF16_NP = ml_dtypes.bfloat16


# ----------------------------------------------------------------------------
# Host-side preparation
# ----------------------------------------------------------------------------

def _build_tmat(w2, b2):
    """Fixed tensor T[f, o*18+k]: contraction of F (128) against (k: 18, o: 64)."""
    alpha = 1.0 / np.sqrt(2 * MUL)
    s3 = 1.0 / np.sqrt(3.0)
    W2k = np.asarray(w2, np.float64).reshape(16, 4, MUL, MUL)  # [k, path, u, w]
    B2 = np.asarray(b2, np.float64).reshape(4, MUL, MUL)       # [path, u, w]
    T = np.zeros((128, NO, NK), np.float64)                    # [f, o, k]
    u = np.arange(MUL)
    for k in range(17):
        Wp = W2k[k] if k < 16 else B2                          # [path, u, w]
        for w in range(MUL):
            # path0: z0 rows (f = u), o = w
            T[u, w, k] += alpha * Wp[0, u, w]
            # path1: z1 rows (f = 112+u), o = w
            T[112 + u, w, k] += alpha * s3 * Wp[1, u, w]
            for i in range(3):
                o1 = MUL + w * 3 + i
                # path2: q_i rows (f = 16+16i+u)
                T[16 + 16 * i + u, o1, k] += alpha * Wp[2, u, w]
                # path3: r_i rows (f = 64+16i+u)
                T[64 + 16 * i + u, o1, k] += alpha * Wp[3, u, w]
    return T.reshape(128, NA).astype(BF16_NP)


def _prepare(inputs):
    node_attr = np.asarray(inputs["node_attr"], np.float32)
    ei = np.asarray(inputs["edge_index"])
    ea = np.asarray(inputs["edge_attr"], np.float32)
    sh = np.asarray(inputs["edge_sh"], np.float32)
    w1 = np.asarray(inputs["w1"], np.float32)
    b1 = np.asarray(inputs["b1"], np.float32)

    src = ei[0].astype(np.int64)
    dst = ei[1].astype(np.int64)
    core = dst // NR
    wloc = (dst - core * NR) // 128

    cnt = np.zeros((NCORES, NWIN), np.int64)
    np.add.at(cnt, (core, wloc), 1)
    tws = np.maximum(1, -(-cnt // 128)).max(axis=0)  # [NWIN] tiles per window
    T = int(tws.sum())
    tile_base = np.zeros(NWIN, np.int64)
    tile_base[1:] = np.cumsum(tws)[:-1]

    src_h = np.zeros((NCORES, T, 128, 1), np.int32)
    dst_h = np.full((NCORES, T, 128, 1), PAD_DST, np.float32)
    sh_h = np.zeros((NCORES, T, 128, 4), BF16_NP)
    eaT_h = np.zeros((NCORES, T, 17, 128), np.float32)
    eaT_h[:, :, 16, :] = 1.0  # bias row (b1 enters h-matmul via this)

    order = np.lexsort((dst, core))
    so_src, so_dst, so_core, so_w = src[order], dst[order], core[order], wloc[order]
    so_ea, so_sh = ea[order], sh[order]

    # group boundaries per (core, window)
    for c in range(NCORES):
        cmask = np.flatnonzero(so_core == c)
        if len(cmask) == 0:
            continue
        lo = cmask[0]
        for w in range(NWIN):
            n = cnt[c, w]
            if n == 0:
                continue
            sel = slice(lo, lo + n)
            lo += n
            base = tile_base[w]
            for j in range(0, n, 128):
                t = base + j // 128
                m = min(128, n - j)
                s2 = slice(sel.start + j, sel.start + j + m)
                src_h[c, t, :m, 0] = so_src[s2]
                dst_h[c, t, :m, 0] = (so_dst[s2] - (c * NR + w * 128)).astype(np.float32)
                sh_h[c, t, :m, :] = so_sh[s2].astype(BF16_NP)
                eaT_h[c, t, :16, :m] = so_ea[s2].T

    node_bf16 = node_attr.astype(BF16_NP)
    w1b = np.concatenate([w1, b1[None, :]], axis=0).astype(np.float32)  # [17, 16]
    tmat = _build_tmat(inputs["w2"], inputs["b2"])

    in_maps = []
    for c in range(NCORES):
        in_maps.append({
            "node": node_bf16,
            "srcd": src_h[c],
            "dstd": dst_h[c],
            "shd": sh_h[c],
            "ead": eaT_h[c],
            "w1b": w1b,
            "tmat": tmat,
        })
    return tuple(int(x) for x in tws), in_maps


# ----------------------------------------------------------------------------
# Bass program
# ----------------------------------------------------------------------------

def build_nc(tws):
    T = int(sum(tws))
    nc = bacc.Bacc("TRN2", target_bir_lowering=False, debug=False)

    node = nc.dram_tensor("node", [N_NODES, 64], BF16, kind="ExternalInput")
    srcd = nc.dram_tensor("srcd", [T, 128, 1], I32, kind="ExternalInput")
    dstd = nc.dram_tensor("dstd", [T, 128, 1], F32, kind="ExternalInput")
    shd = nc.dram_tensor("shd", [T, 128, 4], BF16, kind="ExternalInput")
    ead = nc.dram_tensor("ead", [T, 17, 128], F32, kind="ExternalInput")
    w1bd = nc.dram_tensor("w1b", [17, 16], F32, kind="ExternalInput")
    tmatd = nc.dram_tensor("tmat", [128, NA], BF16, kind="ExternalInput")
    outd = nc.dram_tensor("out", [NR, 64], F32, kind="ExternalOutput")

    MULT = mybir.AluOpType.mult
    ADD = mybir.AluOpType.add
    MAX = mybir.AluOpType.max
    ISEQ = mybir.AluOpType.is_equal
    AXX = mybir.AxisListType.X
    SIGMOID = mybir.ActivationFunctionType.Sigmoid

    with tile.TileContext(nc) as tc, ExitStack() as ctx:
        consts = ctx.enter_context(tc.tile_pool(name="consts", bufs=1))
        # constants
        tmat_s = consts.tile([128, NA], BF16)
        nc.sync.dma_start(tmat_s[:], tmatd[:])
        w1b_s = consts.tile([17, 16], F32)
        nc.sync.dma_start(w1b_s[:], w1bd[:])
        ident = consts.tile([128, 128], BF16)
        make_identity(nc, ident[:])
        iota_i = consts.tile([128, 128], I32)
        nc.gpsimd.iota(iota_i[:], pattern=[[1, 128]], base=0, channel_multiplier=0)
        iota_f = consts.tile([128, 128], F32)
        nc.vector.tensor_copy(iota_f[:], iota_i[:])

        # pools
        inp = ctx.enter_context(tc.tile_pool(name="inp", bufs=4))
        xp = ctx.enter_context(tc.tile_pool(name="xp", bufs=3))
        fp = ctx.enter_context(tc.tile_pool(name="fp", bufs=3))
        scr = ctx.enter_context(tc.tile_pool(name="scr", bufs=3))
        ftp = ctx.enter_context(tc.tile_pool(name="ftp", bufs=3))
        hpool = ctx.enter_context(tc.tile_pool(name="hpool", bufs=3))
        apool = ctx.enter_context(tc.tile_pool(name="apool", bufs=2))
        ppool = ctx.enter_context(tc.tile_pool(name="ppool", bufs=2))
        ohp = ctx.enter_context(tc.tile_pool(name="ohp", bufs=3))
        wfin = ctx.enter_context(tc.tile_pool(name="wfin", bufs=2))
        outp = ctx.enter_context(tc.tile_pool(name="outp", bufs=2))

        ps_ft = ctx.enter_context(tc.tile_pool(name="ps_ft", bufs=1, space="PSUM"))
        ps_h = ctx.enter_context(tc.tile_pool(name="ps_h", bufs=1, space="PSUM"))
        ps_a = ctx.enter_context(tc.tile_pool(name="ps_a", bufs=1, space="PSUM"))
        ps_acc = ctx.enter_context(tc.tile_pool(name="ps_acc", bufs=1, space="PSUM"))

        ti = 0
        for w in range(NWIN):
            tw = int(tws[w])
            acc = ps_acc.tile([128, NA + 1], F32, space="PSUM")
            for t in range(tw):
                # ---- loads ----
                src_t = inp.tile([128, 1], I32, tag="src")
                nc.sync.dma_start(src_t[:], srcd[ti])
                dst_t = inp.tile([128, 1], F32, tag="dst")
                nc.sync.dma_start(dst_t[:], dstd[ti])
                sh_t = inp.tile([128, 4], BF16, tag="sh")
                nc.sync.dma_start(sh_t[:], shd[ti])
                ea_t = inp.tile([17, 128], F32, tag="ea")
                nc.sync.dma_start(ea_t[:], ead[ti])
                x_t = xp.tile([128, 64], BF16)
                nc.gpsimd.indirect_dma_start(
                    out=x_t[:], out_offset=None, in_=node[:],
                    in_offset=IndirectOffsetOnAxis(ap=src_t[:, :1], axis=0),
                )

                # ---- F build (DVE) ----
                fb = fp.tile([128, 128], BF16)
                # [z0|q0|q1|q2]: x0 (bcast over 4 s) * sh[:, s]
                nc.vector.tensor_tensor(
                    out=fb[:, 0:64].rearrange("p (s u) -> p s u", s=4),
                    in0=x_t[:, None, 0:16].to_broadcast([128, 4, 16]),
                    in1=sh_t[:, :, None].to_broadcast([128, 4, 16]),
                    op=MULT,
                )
                # [r0|r1|r2]: x1 (i-major view) * y0
                nc.vector.tensor_tensor(
                    out=fb[:, 64:112].rearrange("p (i u) -> p i u", i=3),
                    in0=x_t[:, 16:64].rearrange("p (u i) -> p u i", i=3)
                         .rearrange("p u i -> p i u"),
                    in1=sh_t[:, 0:1, None].to_broadcast([128, 3, 16]),
                    op=MULT,
                )
                # z1 = sum_i x1[u,i]*y1[i]
                t3 = scr.tile([128, 48], BF16, tag="t3")
                nc.vector.tensor_tensor(
                    out=t3[:].rearrange("p (u i) -> p u i", i=3),
                    in0=x_t[:, 16:64].rearrange("p (u i) -> p u i", i=3),
                    in1=sh_t[:, None, 1:4].to_broadcast([128, 16, 3]),
                    op=MULT,
                )
                z1f = scr.tile([128, 16], F32, tag="z1f")
                nc.vector.tensor_reduce(
                    out=z1f[:], in_=t3[:].rearrange("p (u i) -> p u i", i=3),
                    axis=AXX, op=ADD,
                )
                nc.vector.tensor_copy(fb[:, 112:128], z1f[:])

                # ---- transpose F ----
                ft_ps = ps_ft.tile([128, 128], BF16, space="PSUM")
                nc.tensor.transpose(ft_ps[:], fb[:], ident[:])
                ft = ftp.tile([128, 128], BF16)
                nc.scalar.copy(ft[:], ft_ps[:])

                # ---- h' = [silu(ea@w1+b1), 1, 0] ----
                h_ps = ps_h.tile([128, 16], F32, space="PSUM")
                nc.tensor.matmul(h_ps[:], lhsT=ea_t[:], rhs=w1b_s[:],
                                 start=True, stop=True)
                h_t = hpool.tile([128, NK], BF16)
                h_sig = hpool.tile([128, 16], BF16, tag="hsig")
                nc.scalar.activation(h_sig[:], h_ps[:], SIGMOID)
                nc.vector.tensor_tensor(out=h_t[:, 0:16], in0=h_ps[:],
                                        in1=h_sig[:], op=MULT)
                nc.vector.memset(h_t[:, 16:17], 1.0)
                nc.vector.memset(h_t[:, 17:18], 0.0)

                # ---- A = FT.T @ Tmat ----
                a_ps = ps_a.tile([128, NA], F32, space="PSUM")
                for lo, hi in ((0, 512), (512, 1024), (1024, NA)):
                    nc.tensor.matmul(a_ps[:, lo:hi], lhsT=ft[:],
                                     rhs=tmat_s[:, lo:hi], start=True, stop=True)
                a_sb = apool.tile([128, NA], BF16)
                nc.scalar.copy(a_sb[:], a_ps[:])

                # ---- P = A * h'(k) ----
                p_t = ppool.tile([128, NA + 1], BF16)
                nc.vector.tensor_tensor(
                    out=p_t[:, 0:NA].rearrange("p (o k) -> p o k", k=NK),
                    in0=a_sb[:].rearrange("p (o k) -> p o k", k=NK),
                    in1=h_t[:, None, :].to_broadcast([128, NO, NK]),
                    op=MULT,
                )
                nc.vector.memset(p_t[:, NA:NA + 1], 1.0)

                # ---- one-hot of dst, scatter ----
                oh = ohp.tile([128, 128], BF16)
                nc.vector.tensor_scalar(
                    out=oh[:], in0=iota_f[:], scalar1=dst_t[:, 0:1],
                    scalar2=None, op0=ISEQ,
                )
                first, last = (t == 0), (t == tw - 1)
                for lo, hi in ((0, 512), (512, 1024), (1024, NA + 1)):
                    nc.tensor.matmul(acc[:, lo:hi], lhsT=oh[:], rhs=p_t[:, lo:hi],
                                     start=first, stop=last)
                ti += 1

            # ---- window finalize ----
            acc_sb = wfin.tile([128, NA + 1], F32, tag="acc_sb")
            nc.scalar.copy(acc_sb[:], acc[:])
            sums = wfin.tile([128, 64], F32, tag="sums")
            nc.vector.tensor_reduce(
                out=sums[:], in_=acc_sb[:, 0:NA].rearrange("p (o k) -> p o k", k=NK),
                axis=AXX, op=ADD,
            )
            cnt_t = wfin.tile([128, 1], F32, tag="cnt")
            nc.vector.tensor_scalar(out=cnt_t[:], in0=acc_sb[:, NA:NA + 1],
                                    scalar1=1.0, scalar2=None, op0=MAX)
            rec = wfin.tile([128, 1], F32, tag="rec")
            nc.vector.reciprocal(rec[:], cnt_t[:])
            ow = outp.tile([128, 64], F32)
            nc.vector.tensor_scalar(out=ow[:], in0=sums[:], scalar1=rec[:, 0:1],
                                    scalar2=None, op0=MULT)
            nc.sync.dma_start(outd[w * 128:(w + 1) * 128, :], ow[:])

    nc.compile()
    return nc


_CACHE = {}


def _get_nc(tws):
    if tws not in _CACHE:
        _CACHE[tws] = build_nc(tws)
    return _CACHE[tws]


def kernel(**inputs) -> np.ndarray:
    tws, in_maps = _prepare(inputs)
    nc = _get_nc(tws)
    res = run_bass_kernel_spmd(nc, in_maps, core_ids=list(range(NCORES)))
    outs = [np.asarray(r["out"], np.float32) for r in res.results]
    full = np.concatenate(outs, axis=0)[:N_NODES]
    return full



# revision 5
# speedup vs baseline: 4.8927x; 4.8927x over previous
"""Trainium2 Bass kernel for nn_Conv_12962211300035 (e3nn-style GNN conv).

Math (per edge e, src s, dst d):
  x = node_attr[s]; x0 = x[:16], x1 = x[16:].reshape(16,3)
  h = silu(edge_attr @ w1 + b1); W = (h @ w2 + b2).reshape(4,16,16)
  out0[w]   = sum_u (x0*y0)[u] W0[u,w] + (1/sqrt3) sum_u (sum_i x1[u,i]y1[i]) W1[u,w]
  out1[w,i] = sum_u (x0[u]*y1[i]) W2[u,w] + sum_u (x1[u,i]*y0) W3[u,w]
  out_ij = alpha * [out0 | out1.reshape(48)]     (alpha = 1/sqrt(32))
  node output = scatter-mean of out_ij over dst.

Kernel strategy (8 NeuronCores, SPMD):
  - Nodes range-partitioned: core c owns nodes [c*1664, (c+1)*1664); edges are
    assigned to the core owning their dst, grouped into 13 windows of 128
    nodes, tiled into 128-edge tiles (window tile counts uniform across
    cores; padding edges have out-of-range dst -> one-hot 0).
  - Host->device traffic is minimized: edges ship as bf16 (eaT tiles,
    sh+dst packed), the node table and the folded weight tensor ship
    sharded once and are AllGathered on device, donated output buffers are
    created on device, and the output returns as one replicated bf16 array.
  - Per tile: gather x (indirect DMA from the gathered node table, bf16),
    build a 128-dim geometry feature F on DVE, transpose via PE,
    A = F.T-matmul against a fixed [128, 64*18] tensor (w2/b2/CG folded in),
    P = A * h'-broadcast (DVE), scatter-matmul with a one-hot of dst into a
    per-window PSUM accumulator [128 nodes, 64*18+1].
  - Per window: copy accumulator out, reduce over k (18), divide by counts.
  - Final: AllGather the per-core [1664, 64] block into the full output.
"""

import sys

sys.path.insert(0, "/opt/trn_rl_repo")

import numpy as np
import ml_dtypes
from contextlib import ExitStack

import jax
from jax.sharding import Mesh, PartitionSpec, NamedSharding
from jax.experimental.shard_map import shard_map

import concourse.bass as bass
import concourse.bacc as bacc
import concourse.tile as tile
import concourse.mybir as mybir
from concourse.bass import IndirectOffsetOnAxis
from concourse.bass2jax import install_neuronx_cc_hook, _bass_exec_p, partition_id_tensor
from concourse.masks import make_identity

MUL = 16
N_NODES = 12500
N_EDGES = 200000
EA = 16
NCORES = 8
NWIN = 13          # windows of 128 nodes per core
NR = NWIN * 128    # 1664 nodes per core (8*1664 = 13312 >= 12500)
NFULL = NCORES * NR
NP = 1568          # node-table rows per core for the AllGather (8*1568 = 12544)
NPAD = NCORES * NP
NK = 18            # h' slots: 16 h + 1 bias + 1 zero pad (even for DVE 2x)
NO = 64            # outputs per edge
NA = NO * NK       # 1152 A/P columns
PAD_DST = 999.0

F32 = mybir.dt.float32
BF16 = mybir.dt.bfloat16
I32 = mybir.dt.int32
BF16_NP = ml_dtypes.bfloat16


# ----------------------------------------------------------------------------
# Host-side preparation
# ----------------------------------------------------------------------------

def _build_tmat(w2, b2):
    """Fixed tensor T[f, o*18+k]: contraction of F (128) against (k: 18, o: 64)."""
    alpha = 1.0 / np.sqrt(2 * MUL)
    s3 = 1.0 / np.sqrt(3.0)
    W2k = np.asarray(w2, np.float64).reshape(16, 4, MUL, MUL)  # [k, path, u, w]
    B2 = np.asarray(b2, np.float64).reshape(4, MUL, MUL)       # [path, u, w]
    T = np.zeros((128, NO, NK), np.float64)                    # [f, o, k]
    u = np.arange(MUL)
    for k in range(17):
        Wp = W2k[k] if k < 16 else B2                          # [path, u, w]
        for w in range(MUL):
            # path0: z0 rows (f = u), o = w
            T[u, w, k] += alpha * Wp[0, u, w]
            # path1: z1 rows (f = 112+u), o = w
            T[112 + u, w, k] += alpha * s3 * Wp[1, u, w]
            for i in range(3):
                o1 = MUL + w * 3 + i
                # path2: q_i rows (f = 16+16i+u)
                T[16 + 16 * i + u, o1, k] += alpha * Wp[2, u, w]
                # path3: r_i rows (f = 64+16i+u)
                T[64 + 16 * i + u, o1, k] += alpha * Wp[3, u, w]
    return T.reshape(128, NA).astype(BF16_NP)


def _prepare(inputs):
    node_attr = np.asarray(inputs["node_attr"], np.float32)
    ei = np.asarray(inputs["edge_index"])
    ea = np.asarray(inputs["edge_attr"], np.float32)
    sh = np.asarray(inputs["edge_sh"], np.float32)
    w1 = np.asarray(inputs["w1"], np.float32)
    b1 = np.asarray(inputs["b1"], np.float32)

    src = ei[0].astype(np.int64)
    dst = ei[1].astype(np.int64)
    core = dst // NR
    wloc = (dst - core * NR) // 128
    gid = core * NWIN + wloc

    cnt = np.bincount(gid, minlength=NCORES * NWIN).reshape(NCORES, NWIN)
    tws = np.maximum(1, -(-cnt // 128)).max(axis=0)  # [NWIN] tiles per window
    T = int(tws.sum())
    tile_base = np.zeros(NWIN, np.int64)
    tile_base[1:] = np.cumsum(tws)[:-1]

    order = np.argsort(gid, kind="stable")
    g_sorted = gid[order]
    gstart = np.zeros(NCORES * NWIN + 1, np.int64)
    gstart[1:] = np.cumsum(cnt.ravel())
    j = np.arange(N_EDGES, dtype=np.int64) - gstart[g_sorted]
    cr, wl = core[order], wloc[order]
    tloc = tile_base[wl] + j // 128
    slot = j % 128
    flat = (cr * T + tloc) * 128 + slot        # position in [NCORES*T, 128]

    src_h = np.zeros(NCORES * T * 128, np.int32)
    src_h[flat] = src[order]
    shdst = np.zeros((NCORES * T * 128, 5), BF16_NP)
    shdst[:, 4] = PAD_DST
    shdst[flat, :4] = sh[order].astype(BF16_NP)
    shdst[flat, 4] = (dst[order] - (cr * NR + wl * 128)).astype(BF16_NP)
    ea_p = np.zeros((NCORES * T * 128, EA), BF16_NP)
    ea_p[flat] = ea[order].astype(BF16_NP)
    eaT = np.ascontiguousarray(
        ea_p.reshape(NCORES * T, 128, EA).transpose(0, 2, 1))

    node_pad = np.zeros((NPAD, 64), BF16_NP)
    node_pad[:N_NODES] = node_attr.astype(BF16_NP)
    w1b = np.concatenate([w1, b1[None, :]], axis=0).astype(BF16_NP)  # [17, 16]
    arrays = {
        "node": node_pad,                                   # -> [NP, 64] per core
        "tmat": _build_tmat(inputs["w2"], inputs["b2"]),    # -> [16, NA] per core
        "w1b": np.tile(w1b, (NCORES, 1)),                   # -> [17, 16] per core
        "srcd": src_h.reshape(NCORES * T, 128, 1),
        "shdst": shdst.reshape(NCORES * T, 128, 5),
        "ead": eaT,                                         # [NCORES*T, 16, 128]
    }
    return tuple(int(x) for x in tws), arrays


# ----------------------------------------------------------------------------
# Bass program
# ----------------------------------------------------------------------------

def build_nc(tws):
    T = int(sum(tws))
    nc = bacc.Bacc("TRN2", target_bir_lowering=False, debug=False)

    nodesh = nc.dram_tensor("node", [NP, 64], BF16, kind="ExternalInput")
    tmatd = nc.dram_tensor("tmat", [128 // NCORES, NA], BF16, kind="ExternalInput")
    w1bd = nc.dram_tensor("w1b", [17, 16], BF16, kind="ExternalInput")
    srcd = nc.dram_tensor("srcd", [T, 128, 1], I32, kind="ExternalInput")
    shdstd = nc.dram_tensor("shdst", [T, 128, 5], BF16, kind="ExternalInput")
    ead = nc.dram_tensor("ead", [T, 16, 128], BF16, kind="ExternalInput")
    outd = nc.dram_tensor("out", [NFULL, 64], BF16, kind="ExternalOutput")

    # internal DRAM for collectives
    cc_nin = nc.dram_tensor("cc_nin", [NP, 64], BF16, kind="Internal")
    cc_node = nc.dram_tensor("cc_node", [NPAD, 64], BF16, kind="Internal",
                             addr_space="Shared")
    cc_tin = nc.dram_tensor("cc_tin", [128 // NCORES, NA], BF16, kind="Internal")
    cc_tmat = nc.dram_tensor("cc_tmat", [128, NA], BF16, kind="Internal",
                             addr_space="Shared")
    cc_oin = nc.dram_tensor("cc_oin", [NR, 64], BF16, kind="Internal")
    cc_oout = nc.dram_tensor("cc_oout", [NFULL, 64], BF16, kind="Internal",
                             addr_space="Shared")

    MULT = mybir.AluOpType.mult
    ADD = mybir.AluOpType.add
    MAX = mybir.AluOpType.max
    ISEQ = mybir.AluOpType.is_equal
    BYPASS = mybir.AluOpType.bypass
    AXX = mybir.AxisListType.X
    SIGMOID = mybir.ActivationFunctionType.Sigmoid
    RG = [list(range(NCORES))]

    with tile.TileContext(nc) as tc, ExitStack() as ctx:
        # gather replicated constants (node table, tmat) across cores
        nc.gpsimd.dma_start(cc_nin[:], nodesh[:])
        nc.gpsimd.collective_compute("AllGather", BYPASS, replica_groups=RG,
                                     ins=[cc_nin.ap()], outs=[cc_node.ap()])
        nc.gpsimd.dma_start(cc_tin[:], tmatd[:])
        nc.gpsimd.collective_compute("AllGather", BYPASS, replica_groups=RG,
                                     ins=[cc_tin.ap()], outs=[cc_tmat.ap()])

        consts = ctx.enter_context(tc.tile_pool(name="consts", bufs=1))
        tmat_s = consts.tile([128, NA], BF16)
        nc.sync.dma_start(tmat_s[:], cc_tmat[:])
        w1_s = consts.tile([16, 16], BF16)
        nc.sync.dma_start(w1_s[:], w1bd[0:16, :])
        b1_s = consts.tile([1, 16], BF16)
        nc.sync.dma_start(b1_s[:], w1bd[16:17, :])
        ident = consts.tile([128, 128], BF16)
        make_identity(nc, ident[:])
        iota_i = consts.tile([128, 128], I32)
        nc.gpsimd.iota(iota_i[:], pattern=[[1, 128]], base=0, channel_multiplier=0)
        iota_f = consts.tile([128, 128], BF16)
        nc.vector.tensor_copy(iota_f[:], iota_i[:])
        ones_r = consts.tile([1, 128], BF16)
        nc.vector.memset(ones_r[:], 1.0)

        # pools
        inp = ctx.enter_context(tc.tile_pool(name="inp", bufs=4))
        xp = ctx.enter_context(tc.tile_pool(name="xp", bufs=3))
        fp = ctx.enter_context(tc.tile_pool(name="fp", bufs=3))
        scr = ctx.enter_context(tc.tile_pool(name="scr", bufs=3))
        ftp = ctx.enter_context(tc.tile_pool(name="ftp", bufs=3))
        hpool = ctx.enter_context(tc.tile_pool(name="hpool", bufs=3))
        apool = ctx.enter_context(tc.tile_pool(name="apool", bufs=2))
        ppool = ctx.enter_context(tc.tile_pool(name="ppool", bufs=2))
        ohp = ctx.enter_context(tc.tile_pool(name="ohp", bufs=3))
        wfin = ctx.enter_context(tc.tile_pool(name="wfin", bufs=2))
        outp = ctx.enter_context(tc.tile_pool(name="outp", bufs=2))

        ps_ft = ctx.enter_context(tc.tile_pool(name="ps_ft", bufs=1, space="PSUM"))
        ps_h = ctx.enter_context(tc.tile_pool(name="ps_h", bufs=1, space="PSUM"))
        ps_a = ctx.enter_context(tc.tile_pool(name="ps_a", bufs=1, space="PSUM"))
        ps_acc = ctx.enter_context(tc.tile_pool(name="ps_acc", bufs=1, space="PSUM"))

        ti = 0
        for w in range(NWIN):
            tw = int(tws[w])
            acc = ps_acc.tile([128, NA + 1], F32, space="PSUM")
            for t in range(tw):
                # ---- loads ----
                src_t = inp.tile([128, 1], I32, tag="src")
                nc.sync.dma_start(src_t[:], srcd[ti])
                sd_t = inp.tile([128, 5], BF16, tag="shdst")
                nc.sync.dma_start(sd_t[:], shdstd[ti])
                ea_t = inp.tile([16, 128], BF16, tag="ea")
                nc.sync.dma_start(ea_t[:], ead[ti])
                x_t = xp.tile([128, 64], BF16)
                nc.gpsimd.indirect_dma_start(
                    out=x_t[:], out_offset=None, in_=cc_node[:],
                    in_offset=IndirectOffsetOnAxis(ap=src_t[:, :1], axis=0),
                )

                # ---- F build (DVE) ----
                fb = fp.tile([128, 128], BF16)
                # [z0|q0|q1|q2]: x0 (bcast over 4 s) * sh[:, s]
                nc.vector.tensor_tensor(
                    out=fb[:, 0:64].rearrange("p (s u) -> p s u", s=4),
                    in0=x_t[:, None, 0:16].to_broadcast([128, 4, 16]),
                    in1=sd_t[:, 0:4, None].to_broadcast([128, 4, 16]),
                    op=MULT,
                )
                # [r0|r1|r2]: x1 (i-major view) * y0
                nc.vector.tensor_tensor(
                    out=fb[:, 64:112].rearrange("p (i u) -> p i u", i=3),
                    in0=x_t[:, 16:64].rearrange("p (u i) -> p u i", i=3)
                         .rearrange("p u i -> p i u"),
                    in1=sd_t[:, 0:1, None].to_broadcast([128, 3, 16]),
                    op=MULT,
                )
                # z1 = sum_i x1[u,i]*y1[i]
                t3 = scr.tile([128, 48], BF16, tag="t3")
                nc.vector.tensor_tensor(
                    out=t3[:].rearrange("p (u i) -> p u i", i=3),
                    in0=x_t[:, 16:64].rearrange("p (u i) -> p u i", i=3),
                    in1=sd_t[:, None, 1:4].to_broadcast([128, 16, 3]),
                    op=MULT,
                )
                z1f = scr.tile([128, 16], F32, tag="z1f")
                nc.vector.tensor_reduce(
                    out=z1f[:], in_=t3[:].rearrange("p (u i) -> p u i", i=3),
                    axis=AXX, op=ADD,
                )
                nc.vector.tensor_copy(fb[:, 112:128], z1f[:])

                # ---- transpose F ----
                ft_ps = ps_ft.tile([128, 128], BF16, space="PSUM")
                nc.tensor.transpose(ft_ps[:], fb[:], ident[:])
                ft = ftp.tile([128, 128], BF16)
                nc.scalar.copy(ft[:], ft_ps[:])

                # ---- h' = [silu(ea@w1+b1), 1, 0] ----
                h_ps = ps_h.tile([128, 16], F32, space="PSUM")
                nc.tensor.matmul(h_ps[:], lhsT=ea_t[:], rhs=w1_s[:],
                                 start=True, stop=False)
                nc.tensor.matmul(h_ps[:], lhsT=ones_r[:], rhs=b1_s[:],
                                 start=False, stop=True)
                h_t = hpool.tile([128, NK], BF16)
                h_sig = hpool.tile([128, 16], BF16, tag="hsig")
                nc.scalar.activation(h_sig[:], h_ps[:], SIGMOID)
                nc.vector.tensor_tensor(out=h_t[:, 0:16], in0=h_ps[:],
                                        in1=h_sig[:], op=MULT)
                nc.vector.memset(h_t[:, 16:17], 1.0)
                nc.vector.memset(h_t[:, 17:18], 0.0)

                # ---- A = FT.T @ Tmat ----
                a_ps = ps_a.tile([128, NA], F32, space="PSUM")
                for lo, hi in ((0, 512), (512, 1024), (1024, NA)):
                    nc.tensor.matmul(a_ps[:, lo:hi], lhsT=ft[:],
                                     rhs=tmat_s[:, lo:hi], start=True, stop=True)
                a_sb = apool.tile([128, NA], BF16)
                nc.scalar.copy(a_sb[:], a_ps[:])

                # ---- P = A * h'(k) ----
                p_t = ppool.tile([128, NA + 1], BF16)
                nc.vector.tensor_tensor(
                    out=p_t[:, 0:NA].rearrange("p (o k) -> p o k", k=NK),
                    in0=a_sb[:].rearrange("p (o k) -> p o k", k=NK),
                    in1=h_t[:, None, :].to_broadcast([128, NO, NK]),
                    op=MULT,
                )
                nc.vector.memset(p_t[:, NA:NA + 1], 1.0)

                # ---- one-hot of dst, scatter ----
                dst_f = scr.tile([128, 1], F32, tag="dstf")
                nc.vector.tensor_copy(dst_f[:], sd_t[:, 4:5])
                oh = ohp.tile([128, 128], BF16)
                nc.vector.tensor_scalar(
                    out=oh[:], in0=iota_f[:], scalar1=dst_f[:, 0:1],
                    scalar2=None, op0=ISEQ,
                )
                first, last = (t == 0), (t == tw - 1)
                for lo, hi in ((0, 512), (512, 1024), (1024, NA + 1)):
                    nc.tensor.matmul(acc[:, lo:hi], lhsT=oh[:], rhs=p_t[:, lo:hi],
                                     start=first, stop=last)
                ti += 1

            # ---- window finalize ----
            acc_sb = wfin.tile([128, NA + 1], F32, tag="acc_sb")
            nc.scalar.copy(acc_sb[:], acc[:])
            sums = wfin.tile([128, 64], F32, tag="sums")
            nc.vector.tensor_reduce(
                out=sums[:], in_=acc_sb[:, 0:NA].rearrange("p (o k) -> p o k", k=NK),
                axis=AXX, op=ADD,
            )
            cnt_t = wfin.tile([128, 1], F32, tag="cnt")
            nc.vector.tensor_scalar(out=cnt_t[:], in0=acc_sb[:, NA:NA + 1],
                                    scalar1=1.0, scalar2=None, op0=MAX)
            rec = wfin.tile([128, 1], F32, tag="rec")
            nc.vector.reciprocal(rec[:], cnt_t[:])
            ow = outp.tile([128, 64], BF16)
            nc.vector.tensor_scalar(out=ow[:], in0=sums[:], scalar1=rec[:, 0:1],
                                    scalar2=None, op0=MULT)
            nc.sync.dma_start(cc_oin[w * 128:(w + 1) * 128, :], ow[:])

        # ---- gather full output onto every core ----
        nc.gpsimd.collective_compute("AllGather", BYPASS, replica_groups=RG,
                                     ins=[cc_oin.ap()], outs=[cc_oout.ap()])
        nc.sync.dma_start(outd[:], cc_oout[:])

    nc.compile()
    return nc


# ----------------------------------------------------------------------------
# Cached jit runner
# ----------------------------------------------------------------------------

_CACHE = {}


def _get_runner(tws):
    if tws in _CACHE:
        return _CACHE[tws]

    nc = build_nc(tws)
    install_neuronx_cc_hook()

    partition_name = nc.partition_id_tensor.name if nc.partition_id_tensor else None
    in_names, out_names, out_avals = [], [], []
    for alloc in nc.m.functions[0].allocations:
        if not isinstance(alloc, mybir.MemoryLocationSet):
            continue
        if alloc.kind not in ("ExternalInput", "ExternalOutput"):
            continue
        name = alloc.memorylocations[0].name
        if alloc.kind == "ExternalInput":
            if name != partition_name:
                in_names.append(name)
        else:
            out_names.append(name)
            out_avals.append(jax.core.ShapedArray(tuple(alloc.tensor_shape),
                                                  mybir.dt.np(alloc.dtype)))
    n_params = len(in_names)
    in_names_all = in_names + out_names + ([partition_name] if partition_name else [])

    def _body(*args):
        operands = list(args)
        if partition_name is not None:
            operands.append(partition_id_tensor())
        outs = _bass_exec_p.bind(
            *operands, out_avals=tuple(out_avals), in_names=tuple(in_names_all),
            out_names=tuple(out_names), lowering_input_output_aliases=(),
            sim_require_finite=True, sim_require_nnan=True, nc=nc)
        return tuple(outs)

    devices = jax.devices()[:NCORES]
    mesh = Mesh(np.asarray(devices), ("core",))
    P = PartitionSpec
    in_specs = (P("core"),) * n_params + (P(None),) * len(out_names)
    out_specs = (P(None),) * len(out_names)
    sharded = jax.jit(
        shard_map(_body, mesh=mesh, in_specs=in_specs, out_specs=out_specs,
                  check_rep=False),
        donate_argnums=tuple(range(n_params, n_params + len(out_names))),
        keep_unused=True,
    )
    repl = NamedSharding(mesh, P(None))
    mkzeros = jax.jit(lambda: jax.numpy.zeros((NFULL, 64), jax.numpy.bfloat16),
                      out_shardings=repl)
    runner = (sharded, mkzeros, in_names)
    _CACHE[tws] = runner
    return runner


def _execute(tws, arrays):
    """Host numpy in -> host numpy out (full transfer + exec + fetch)."""
    sharded, mkzeros, in_names = _get_runner(tws)
    z = mkzeros()
    out = sharded(*[arrays[n] for n in in_names], z)
    return np.asarray(out[0])


def kernel(**inputs) -> np.ndarray:
    tws, arrays = _prepare(inputs)
    full = _execute(tws, arrays)
    return full[:N_NODES].astype(np.float32)


# revision 10
# speedup vs baseline: 8.4207x; 1.7211x over previous
"""Trainium2 Bass kernel for nn_Conv_12962211300035 (e3nn-style GNN conv).

Math (per edge e, src s, dst d):
  x = node_attr[s]; x0 = x[:16], x1 = x[16:].reshape(16,3)
  h = silu(edge_attr @ w1 + b1); W = (h @ w2 + b2).reshape(4,16,16)
  out0[w]   = sum_u (x0*y0)[u] W0[u,w] + (1/sqrt3) sum_u (sum_i x1[u,i]y1[i]) W1[u,w]
  out1[w,i] = sum_u (x0[u]*y1[i]) W2[u,w] + sum_u (x1[u,i]*y0) W3[u,w]
  out_ij = alpha * [out0 | out1.reshape(48)]     (alpha = 1/sqrt(32))
  node output = scatter-mean of out_ij over dst.

Kernel strategy (8 NeuronCores, SPMD):
  - Nodes range-partitioned: core c owns nodes [c*1664, (c+1)*1664); edges are
    assigned to the core owning their dst, grouped into 13 windows of 128
    nodes, tiled into 128-edge tiles (window tile counts uniform across
    cores; padding edges have out-of-range dst -> one-hot 0).
  - Host->device traffic is minimized: edges ship as bf16 (eaT tiles,
    sh+dst packed), the node table and the folded weight tensor ship
    sharded once and are AllGathered on device, donated output buffers are
    created on device, and the output returns as one replicated bf16 array.
  - Per tile: gather x (indirect DMA from the gathered node table, bf16),
    build a 128-dim geometry feature F on DVE, transpose via PE,
    A = F.T-matmul against a fixed [128, 64*18] tensor (w2/b2/CG folded in),
    P = A * h'-broadcast (DVE), scatter-matmul with a one-hot of dst into a
    per-window PSUM accumulator [128 nodes, 64*18+1].
  - Per window: copy accumulator out, reduce over k (18), divide by counts.
  - Final: AllGather the per-core [1664, 64] block into the full output.
"""

import sys

sys.path.insert(0, "/opt/trn_rl_repo")

import numpy as np
import ml_dtypes
from contextlib import ExitStack

import jax
from jax.sharding import Mesh, PartitionSpec, NamedSharding
from jax.experimental.shard_map import shard_map

import concourse.bass as bass
import concourse.bacc as bacc
import concourse.tile as tile
import concourse.mybir as mybir
from concourse.bass import IndirectOffsetOnAxis
from concourse.bass2jax import install_neuronx_cc_hook, _bass_exec_p, partition_id_tensor
from concourse.masks import make_identity

MUL = 16
N_NODES = 12500
N_EDGES = 200000
EA = 16
NCORES = 8
NWIN = 13          # windows of 128 nodes per core
NR = NWIN * 128    # 1664 nodes per core (8*1664 = 13312 >= 12500)
NFULL = NCORES * NR
NP = 1568          # node-table rows per core for the AllGather (8*1568 = 12544)
NPAD = NCORES * NP
NK = 18            # h' slots: 16 h + 1 bias + 1 zero pad (even for DVE 2x)
NO = 64            # outputs per edge
NA = NO * NK       # 1152 A/P columns
PAD_DST = 999.0

F32 = mybir.dt.float32
BF16 = mybir.dt.bfloat16
I32 = mybir.dt.int32
I16 = mybir.dt.int16
I8 = mybir.dt.int8
BF16_NP = ml_dtypes.bfloat16


# ----------------------------------------------------------------------------
# Host-side preparation
# ----------------------------------------------------------------------------

def _build_tmat(w2, b2):
    """Fixed tensor T[f, o*18+k]: contraction of F (128) against (k: 18, o: 64)."""
    alpha = 1.0 / np.sqrt(2 * MUL)
    s3 = 1.0 / np.sqrt(3.0)
    W2k = np.asarray(w2, np.float64).reshape(16, 4, MUL, MUL)  # [k, path, u, w]
    B2 = np.asarray(b2, np.float64).reshape(4, MUL, MUL)       # [path, u, w]
    T = np.zeros((128, NO, NK), np.float64)                    # [f, o, k]
    u = np.arange(MUL)
    for k in range(17):
        Wp = W2k[k] if k < 16 else B2                          # [path, u, w]
        for w in range(MUL):
            # path0: z0 rows (f = u), o = w
            T[u, w, k] += alpha * Wp[0, u, w]
            # path1: z1 rows (f = 112+u), o = w
            T[112 + u, w, k] += alpha * s3 * Wp[1, u, w]
            for i in range(3):
                o1 = MUL + w * 3 + i
                # path2: q_i rows (f = 16+16i+u)
                T[16 + 16 * i + u, o1, k] += alpha * Wp[2, u, w]
                # path3: r_i rows (f = 64+16i+u)
                T[64 + 16 * i + u, o1, k] += alpha * Wp[3, u, w]
    return T.reshape(128, NA).astype(BF16_NP)


def _prepare(inputs):
    node_attr = np.asarray(inputs["node_attr"], np.float32)
    ei = np.asarray(inputs["edge_index"])
    ea = np.asarray(inputs["edge_attr"], np.float32)
    sh = np.asarray(inputs["edge_sh"], np.float32)
    w1 = np.asarray(inputs["w1"], np.float32)
    b1 = np.asarray(inputs["b1"], np.float32)

    src = ei[0].astype(np.int64)
    dst = ei[1].astype(np.int64)
    core = dst // NR
    wloc = (dst - core * NR) // 128
    gid = core * NWIN + wloc

    cnt = np.bincount(gid, minlength=NCORES * NWIN).reshape(NCORES, NWIN)
    tws = np.maximum(1, -(-cnt // 128)).max(axis=0)  # [NWIN] tiles per window
    T = int(tws.sum())
    tile_base = np.zeros(NWIN, np.int64)
    tile_base[1:] = np.cumsum(tws)[:-1]

    order = np.argsort(gid, kind="stable")
    g_sorted = gid[order]
    gstart = np.zeros(NCORES * NWIN + 1, np.int64)
    gstart[1:] = np.cumsum(cnt.ravel())
    j = np.arange(N_EDGES, dtype=np.int64) - gstart[g_sorted]
    cr, wl = core[order], wloc[order]
    tloc = tile_base[wl] + j // 128
    slot = j % 128
    flat = (cr * T + tloc) * 128 + slot        # position in [NCORES*T, 128]

    src_h = np.zeros(NCORES * T * 128, np.int16)
    src_h[flat] = src[order].astype(np.int16)
    shdst = np.zeros((NCORES * T * 128, 5), BF16_NP)
    shdst[:, 4] = PAD_DST
    shdst[flat, :4] = sh[order].astype(BF16_NP)
    shdst[flat, 4] = (dst[order] - (cr * NR + wl * 128)).astype(BF16_NP)

    # edge_attr as int8 with per-feature scales folded into w1
    s_ea = np.abs(ea).max(axis=0) / 127.0                  # [16]
    ea_q = np.clip(np.round(ea / s_ea), -127, 127).astype(np.int8)
    ea_p = np.zeros((NCORES * T * 128, EA), np.int8)
    ea_p[flat] = ea_q[order]
    eaT = np.ascontiguousarray(
        ea_p.reshape(NCORES * T, 128, EA).transpose(0, 2, 1))

    node_pad = np.zeros((NPAD, 64), BF16_NP)
    node_pad[:N_NODES] = node_attr.astype(BF16_NP)
    w1s = (w1 * s_ea[:, None]).astype(BF16_NP)             # fold dequant scales
    w1b = np.concatenate([w1s, b1[None, :].astype(BF16_NP)], axis=0)  # [17, 16]
    arrays = {
        "node": node_pad,                                   # -> [NP, 64] per core
        "tmat": _build_tmat(inputs["w2"], inputs["b2"]),    # -> [16, NA] per core
        "w1b": np.tile(w1b, (NCORES, 1)),                   # -> [17, 16] per core
        "srcd": src_h.reshape(NCORES * T, 128, 1),
        "shdst": shdst.reshape(NCORES * T, 128, 5),
        "ead": eaT,                                         # [NCORES*T, 16, 128]
    }
    return tuple(int(x) for x in tws), arrays


# ----------------------------------------------------------------------------
# Bass program
# ----------------------------------------------------------------------------

def build_nc(tws):
    T = int(sum(tws))
    nc = bacc.Bacc("TRN2", target_bir_lowering=False, debug=False)

    nodesh = nc.dram_tensor("node", [NP, 64], BF16, kind="ExternalInput")
    tmatd = nc.dram_tensor("tmat", [128 // NCORES, NA], BF16, kind="ExternalInput")
    w1bd = nc.dram_tensor("w1b", [17, 16], BF16, kind="ExternalInput")
    srcd = nc.dram_tensor("srcd", [T, 128, 1], I16, kind="ExternalInput")
    shdstd = nc.dram_tensor("shdst", [T, 128, 5], BF16, kind="ExternalInput")
    ead = nc.dram_tensor("ead", [T, 16, 128], I8, kind="ExternalInput")
    outd = nc.dram_tensor("out", [NFULL, 64], BF16, kind="ExternalOutput")

    # internal DRAM for collectives
    cc_nin = nc.dram_tensor("cc_nin", [NP, 64], BF16, kind="Internal")
    cc_node = nc.dram_tensor("cc_node", [NPAD, 64], BF16, kind="Internal",
                             addr_space="Shared")
    cc_tin = nc.dram_tensor("cc_tin", [128 // NCORES, NA], BF16, kind="Internal")
    cc_tmat = nc.dram_tensor("cc_tmat", [128, NA], BF16, kind="Internal",
                             addr_space="Shared")
    cc_oin = nc.dram_tensor("cc_oin", [NR, 64], BF16, kind="Internal")
    cc_oout = nc.dram_tensor("cc_oout", [NFULL, 64], BF16, kind="Internal",
                             addr_space="Shared")

    MULT = mybir.AluOpType.mult
    ADD = mybir.AluOpType.add
    MAX = mybir.AluOpType.max
    ISEQ = mybir.AluOpType.is_equal
    BYPASS = mybir.AluOpType.bypass
    AXX = mybir.AxisListType.X
    SIGMOID = mybir.ActivationFunctionType.Sigmoid
    RG = [list(range(NCORES))]

    with tile.TileContext(nc) as tc, ExitStack() as ctx:
        # gather replicated constants (node table, tmat) across cores
        nc.gpsimd.dma_start(cc_nin[:], nodesh[:])
        nc.gpsimd.collective_compute("AllGather", BYPASS, replica_groups=RG,
                                     ins=[cc_nin.ap()], outs=[cc_node.ap()])
        nc.gpsimd.dma_start(cc_tin[:], tmatd[:])
        nc.gpsimd.collective_compute("AllGather", BYPASS, replica_groups=RG,
                                     ins=[cc_tin.ap()], outs=[cc_tmat.ap()])

        consts = ctx.enter_context(tc.tile_pool(name="consts", bufs=1))
        tmat_s = consts.tile([128, NA], BF16)
        nc.sync.dma_start(tmat_s[:], cc_tmat[:])
        w1_s = consts.tile([16, 16], BF16)
        nc.sync.dma_start(w1_s[:], w1bd[0:16, :])
        b1_s = consts.tile([1, 16], BF16)
        nc.sync.dma_start(b1_s[:], w1bd[16:17, :])
        ident = consts.tile([128, 128], BF16)
        make_identity(nc, ident[:])
        iota_i = consts.tile([128, 128], I32)
        nc.gpsimd.iota(iota_i[:], pattern=[[1, 128]], base=0, channel_multiplier=0)
        iota_f = consts.tile([128, 128], BF16)
        nc.vector.tensor_copy(iota_f[:], iota_i[:])
        ones_r = consts.tile([1, 128], BF16)
        nc.vector.memset(ones_r[:], 1.0)

        # pools
        inp = ctx.enter_context(tc.tile_pool(name="inp", bufs=4))
        xp = ctx.enter_context(tc.tile_pool(name="xp", bufs=3))
        fp = ctx.enter_context(tc.tile_pool(name="fp", bufs=3))
        scr = ctx.enter_context(tc.tile_pool(name="scr", bufs=3))
        ftp = ctx.enter_context(tc.tile_pool(name="ftp", bufs=3))
        hpool = ctx.enter_context(tc.tile_pool(name="hpool", bufs=3))
        apool = ctx.enter_context(tc.tile_pool(name="apool", bufs=2))
        ppool = ctx.enter_context(tc.tile_pool(name="ppool", bufs=2))
        ohp = ctx.enter_context(tc.tile_pool(name="ohp", bufs=3))
        wfin = ctx.enter_context(tc.tile_pool(name="wfin", bufs=2))
        outp = ctx.enter_context(tc.tile_pool(name="outp", bufs=2))

        ps_ft = ctx.enter_context(tc.tile_pool(name="ps_ft", bufs=1, space="PSUM"))
        ps_h = ctx.enter_context(tc.tile_pool(name="ps_h", bufs=1, space="PSUM"))
        ps_a = ctx.enter_context(tc.tile_pool(name="ps_a", bufs=1, space="PSUM"))
        ps_acc = ctx.enter_context(tc.tile_pool(name="ps_acc", bufs=1, space="PSUM"))

        ti = 0
        for w in range(NWIN):
            tw = int(tws[w])
            acc = ps_acc.tile([128, NA + 1], F32, space="PSUM")
            for t in range(tw):
                # ---- loads ----
                src16 = inp.tile([128, 1], I16, tag="src16")
                nc.sync.dma_start(src16[:], srcd[ti])
                src_t = inp.tile([128, 1], I32, tag="src")
                nc.vector.tensor_copy(src_t[:], src16[:])
                sd_t = inp.tile([128, 5], BF16, tag="shdst")
                nc.sync.dma_start(sd_t[:], shdstd[ti])
                ea8_t = inp.tile([16, 128], I8, tag="ea8")
                nc.sync.dma_start(ea8_t[:], ead[ti])
                ea_t = inp.tile([16, 128], BF16, tag="ea")
                nc.vector.tensor_copy(ea_t[:], ea8_t[:])
                x_t = xp.tile([128, 64], BF16)
                nc.gpsimd.indirect_dma_start(
                    out=x_t[:], out_offset=None, in_=cc_node[:],
                    in_offset=IndirectOffsetOnAxis(ap=src_t[:, :1], axis=0),
                )

                # ---- F build (DVE) ----
                fb = fp.tile([128, 128], BF16)
                # [z0|q0|q1|q2]: x0 (bcast over 4 s) * sh[:, s]
                nc.vector.tensor_tensor(
                    out=fb[:, 0:64].rearrange("p (s u) -> p s u", s=4),
                    in0=x_t[:, None, 0:16].to_broadcast([128, 4, 16]),
                    in1=sd_t[:, 0:4, None].to_broadcast([128, 4, 16]),
                    op=MULT,
                )
                # [r0|r1|r2]: x1 (i-major view) * y0
                nc.vector.tensor_tensor(
                    out=fb[:, 64:112].rearrange("p (i u) -> p i u", i=3),
                    in0=x_t[:, 16:64].rearrange("p (u i) -> p u i", i=3)
                         .rearrange("p u i -> p i u"),
                    in1=sd_t[:, 0:1, None].to_broadcast([128, 3, 16]),
                    op=MULT,
                )
                # z1 = sum_i x1[u,i]*y1[i]
                t3 = scr.tile([128, 48], BF16, tag="t3")
                nc.vector.tensor_tensor(
                    out=t3[:].rearrange("p (u i) -> p u i", i=3),
                    in0=x_t[:, 16:64].rearrange("p (u i) -> p u i", i=3),
                    in1=sd_t[:, None, 1:4].to_broadcast([128, 16, 3]),
                    op=MULT,
                )
                z1f = scr.tile([128, 16], F32, tag="z1f")
                nc.vector.tensor_reduce(
                    out=z1f[:], in_=t3[:].rearrange("p (u i) -> p u i", i=3),
                    axis=AXX, op=ADD,
                )
                nc.vector.tensor_copy(fb[:, 112:128], z1f[:])

                # ---- transpose F ----
                ft_ps = ps_ft.tile([128, 128], BF16, space="PSUM")
                nc.tensor.transpose(ft_ps[:], fb[:], ident[:])
                ft = ftp.tile([128, 128], BF16)
                nc.scalar.copy(ft[:], ft_ps[:])

                # ---- h' = [silu(ea@w1+b1), 1, 0] ----
                h_ps = ps_h.tile([128, 16], F32, space="PSUM")
                nc.tensor.matmul(h_ps[:], lhsT=ea_t[:], rhs=w1_s[:],
                                 start=True, stop=False)
                nc.tensor.matmul(h_ps[:], lhsT=ones_r[:], rhs=b1_s[:],
                                 start=False, stop=True)
                h_t = hpool.tile([128, NK], BF16)
                h_sig = hpool.tile([128, 16], BF16, tag="hsig")
                nc.scalar.activation(h_sig[:], h_ps[:], SIGMOID)
                nc.vector.tensor_tensor(out=h_t[:, 0:16], in0=h_ps[:],
                                        in1=h_sig[:], op=MULT)
                nc.vector.memset(h_t[:, 16:17], 1.0)
                nc.vector.memset(h_t[:, 17:18], 0.0)

                # ---- A = FT.T @ Tmat ----
                a_ps = ps_a.tile([128, NA], F32, space="PSUM")
                for lo, hi in ((0, 512), (512, 1024), (1024, NA)):
                    nc.tensor.matmul(a_ps[:, lo:hi], lhsT=ft[:],
                                     rhs=tmat_s[:, lo:hi], start=True, stop=True)
                a_sb = apool.tile([128, NA], BF16)
                nc.scalar.copy(a_sb[:], a_ps[:])

                # ---- P = A * h'(k) ----
                p_t = ppool.tile([128, NA + 1], BF16)
                nc.vector.tensor_tensor(
                    out=p_t[:, 0:NA].rearrange("p (o k) -> p o k", k=NK),
                    in0=a_sb[:].rearrange("p (o k) -> p o k", k=NK),
                    in1=h_t[:, None, :].to_broadcast([128, NO, NK]),
                    op=MULT,
                )
                nc.vector.memset(p_t[:, NA:NA + 1], 1.0)

                # ---- one-hot of dst, scatter ----
                dst_f = scr.tile([128, 1], F32, tag="dstf")
                nc.vector.tensor_copy(dst_f[:], sd_t[:, 4:5])
                oh = ohp.tile([128, 128], BF16)
                nc.vector.tensor_scalar(
                    out=oh[:], in0=iota_f[:], scalar1=dst_f[:, 0:1],
                    scalar2=None, op0=ISEQ,
                )
                first, last = (t == 0), (t == tw - 1)
                for lo, hi in ((0, 512), (512, 1024), (1024, NA + 1)):
                    nc.tensor.matmul(acc[:, lo:hi], lhsT=oh[:], rhs=p_t[:, lo:hi],
                                     start=first, stop=last)
                ti += 1

            # ---- window finalize ----
            acc_sb = wfin.tile([128, NA + 1], F32, tag="acc_sb")
            nc.scalar.copy(acc_sb[:], acc[:])
            sums = wfin.tile([128, 64], F32, tag="sums")
            nc.vector.tensor_reduce(
                out=sums[:], in_=acc_sb[:, 0:NA].rearrange("p (o k) -> p o k", k=NK),
                axis=AXX, op=ADD,
            )
            cnt_t = wfin.tile([128, 1], F32, tag="cnt")
            nc.vector.tensor_scalar(out=cnt_t[:], in0=acc_sb[:, NA:NA + 1],
                                    scalar1=1.0, scalar2=None, op0=MAX)
            rec = wfin.tile([128, 1], F32, tag="rec")
            nc.vector.reciprocal(rec[:], cnt_t[:])
            ow = outp.tile([128, 64], BF16)
            nc.vector.tensor_scalar(out=ow[:], in0=sums[:], scalar1=rec[:, 0:1],
                                    scalar2=None, op0=MULT)
            nc.sync.dma_start(cc_oin[w * 128:(w + 1) * 128, :], ow[:])

        # ---- gather full output onto every core ----
        nc.gpsimd.collective_compute("AllGather", BYPASS, replica_groups=RG,
                                     ins=[cc_oin.ap()], outs=[cc_oout.ap()])
        nc.sync.dma_start(outd[:], cc_oout[:])

    nc.compile()
    return nc


# ----------------------------------------------------------------------------
# Cached jit runner
# ----------------------------------------------------------------------------

_CACHE = {}


def _get_runner(tws):
    if tws in _CACHE:
        return _CACHE[tws]

    nc = build_nc(tws)
    install_neuronx_cc_hook()

    partition_name = nc.partition_id_tensor.name if nc.partition_id_tensor else None
    in_names, out_names, out_avals = [], [], []
    for alloc in nc.m.functions[0].allocations:
        if not isinstance(alloc, mybir.MemoryLocationSet):
            continue
        if alloc.kind not in ("ExternalInput", "ExternalOutput"):
            continue
        name = alloc.memorylocations[0].name
        if alloc.kind == "ExternalInput":
            if name != partition_name:
                in_names.append(name)
        else:
            out_names.append(name)
            out_avals.append(jax.core.ShapedArray(tuple(alloc.tensor_shape),
                                                  mybir.dt.np(alloc.dtype)))
    n_params = len(in_names)
    in_names_all = in_names + out_names + ([partition_name] if partition_name else [])

    def _body(*args):
        operands = list(args)
        if partition_name is not None:
            operands.append(partition_id_tensor())
        outs = _bass_exec_p.bind(
            *operands, out_avals=tuple(out_avals), in_names=tuple(in_names_all),
            out_names=tuple(out_names), lowering_input_output_aliases=(),
            sim_require_finite=True, sim_require_nnan=True, nc=nc)
        return tuple(outs)

    devices = jax.devices()[:NCORES]
    mesh = Mesh(np.asarray(devices), ("core",))
    P = PartitionSpec
    in_specs = (P("core"),) * n_params + (P(None),) * len(out_names)
    out_specs = (P(None),) * len(out_names)
    sharded = jax.jit(
        shard_map(_body, mesh=mesh, in_specs=in_specs, out_specs=out_specs,
                  check_rep=False),
        donate_argnums=tuple(range(n_params, n_params + len(out_names))),
        keep_unused=True,
    )
    repl = NamedSharding(mesh, P(None))
    mkzeros = jax.jit(lambda: jax.numpy.zeros((NFULL, 64), jax.numpy.bfloat16),
                      out_shardings=repl)
    runner = (sharded, mkzeros, in_names)
    _CACHE[tws] = runner
    return runner


_DONOR = [None]


def _execute(tws, arrays):
    """Host numpy in -> host numpy out (full transfer + exec + fetch)."""
    sharded, mkzeros, in_names = _get_runner(tws)
    z = _DONOR[0]
    if z is None or getattr(z, "is_deleted", lambda: True)():
        z = mkzeros()
    out = sharded(*[arrays[n] for n in in_names], z)
    res = np.asarray(out[0])
    _DONOR[0] = out[0]      # fully overwritten by the kernel next call
    return res


def kernel(**inputs) -> np.ndarray:
    tws, arrays = _prepare(inputs)
    full = _execute(tws, arrays)
    return full[:N_NODES].astype(np.float32)
